# revision 1
# baseline (speedup 1.0000x reference)
"""Bass/Trainium2 kernel v2 for 2-layer GAT (nn_GAT_48919677501958).

Contract: kernel(**inputs) takes FULL unsharded numpy inputs, returns the
FULL [10000, 40] float32 output.

v2 strategy (vs v1): edge-major message layout + PE scatter-add.
  - Host: append self-loops, partition nodes into 8 shards x 10 tiles of
    128 dsts. Per tile, list its incoming edges in CSR (dst-major) order,
    padded to a multiple of 128; edge e sits at (partition e%128, group
    e//128). Indices gather rows of the shared node table; pads use row 0
    with local-dst sentinel 255.
  - Device per core:
      Phase A: own-shard H = X@W1 -> shared table haug [10240, 640] bf16
               rows = [h(512, head-interleaved) | alpha_src(8) | pad];
               alpha_dst kept in SBUF (bf16). AllGather haug.
      Phase B per tile: dma_gather edge rows (edges on partitions);
        C2[e,d] one-hot built on DVE (is_equal vs host iota table);
        CT2[d,e] one-hot loaded from host; alpha_dst broadcast to edges
        via PE (CT2^T @ ad); ex = exp(lrelu(as+ad)) on DVE/ACT, written
        into the gathered rows' alpha slot; messages *= ex (DVE, 2x);
        segment-sum via PE: psum[d, 512+8] = sum_g C2_g^T @ rows_g
        (message sum + softmax denominator in one accumulation group);
        y = psum * (1/denom) + b1, ELU.
      Phase C: PE-transpose y, layer-2 matmul -> h2own rows bf16
               [h2(40) | alpha_src2(1) | pad to 128]. AllGather h2.
      Phase D per tile: same edge machinery with the SAME gather indices
        and the kept C2/CT2; single head; out rows [128, 40] f32.
  - Host: concat per-core outputs, inverse-permute rows.
"""

from dataclasses import dataclass, field

import numpy as np

import concourse.bass as bass
import concourse.mybir as mybir
import concourse.tile as tile
from concourse.bass_utils import run_bass_kernel_spmd
from concourse.masks import make_identity

F32 = mybir.dt.float32
F32R = mybir.dt.float32r
BF16 = mybir.dt.bfloat16
I16 = mybir.dt.int16

NEG_SLOPE = 0.2
P = 128


@dataclass
class Cfg:
    n_nodes: int = 10000
    n_cores: int = 8
    tpc: int = 10
    d_in: int = 256
    hid: int = 64
    heads: int = 8
    d_out: int = 40
    ng: list[int] = field(default_factory=list)  # groups per tile (this prog)
    collective: bool = True
    phases: str = "ABCD"

    @property
    def npc(self):
        return self.tpc * P

    @property
    def npad(self):
        return self.n_cores * self.npc

    @property
    def d_hid(self):
        return self.hid * self.heads

    @property
    def rw1(self):  # table-1 row elems: 512 h + 8 alpha_src + pad (256B rule)
        return 640

    @property
    def rw2(self):  # table-2 row elems: 40 h2 + 1 alpha_src2 + pad (256B rule)
        return 128

    @property
    def ng_max(self):
        return max(self.ng)

    @property
    def sum_ng(self):
        return sum(self.ng)


def _wrap_idx(flat: np.ndarray) -> np.ndarray:
    """dma_gather index layout: position i at [i % 16, i // 16], replicated
    across the 8 GpSimd-core stripes of 16 partitions each."""
    assert flat.size % 16 == 0
    w = np.ascontiguousarray(flat.reshape(-1, 16).T).astype(np.int16)
    return np.tile(w, (8, 1))


def preprocess(cfg: Cfg, x, edge_index, W1, att_src1, att_dst1, b1, W2,
               att_src2, att_dst2, b2):
    N = cfg.n_nodes
    src = np.concatenate([np.asarray(edge_index[0], np.int64), np.arange(N)])
    dst = np.concatenate([np.asarray(edge_index[1], np.int64), np.arange(N)])
    deg = np.bincount(dst, minlength=N)

    # CSR by dst
    order_e = np.argsort(dst, kind="stable")
    sorted_src = src[order_e]
    starts = np.zeros(N + 1, np.int64)
    np.cumsum(deg, out=starts[1:])

    # deal degree-sorted nodes round-robin over cores (equal edge loads),
    # then within each core greedily pack tiles toward weighted edge
    # targets: small first/last tiles shrink pipeline ramp and drain.
    node_order = np.argsort(-deg, kind="stable")
    n_tiles = cfg.n_cores * cfg.tpc
    tiles = np.full((n_tiles, P), -1, np.int64)
    wts = np.array([0.42] + [1.13] * (cfg.tpc - 3) + [0.82, 0.42])
    # cap middle-tile size at 18 groups to bound SBUF pools

    for c in range(cfg.n_cores):
        mine = node_order[c::cfg.n_cores]
        tgt = wts / wts.sum() * deg[mine].sum()
        cur = np.zeros(cfg.tpc)
        cnt = np.zeros(cfg.tpc, np.int64)
        cap = 18 * P - 16  # bound ng_max -> bounds SBUF gather pools
        for n in mine:
            free = cnt < P
            ok = free & (cur + deg[n] <= cap)
            pick = ok if ok.any() else free
            t = int(np.argmax(np.where(pick, tgt - cur, -np.inf)))
            tiles[c * cfg.tpc + t, cnt[t]] = n
            cur[t] += deg[n]
            cnt[t] += 1
    core_tiles = [[c * cfg.tpc + t for t in range(cfg.tpc)]
                  for c in range(cfg.n_cores)]

    # edges per (core, tile-rank): SPMD => ng must be identical across cores
    tile_ne = np.zeros((cfg.n_cores, cfg.tpc), np.int64)
    for c in range(cfg.n_cores):
        for t in range(cfg.tpc):
            nodes = tiles[core_tiles[c][t]]
            tile_ne[c, t] = deg[nodes[nodes >= 0]].sum()
    cfg.ng = [int(np.ceil(tile_ne[:, t].max() / P)) for t in range(cfg.tpc)]

    # slot -> node map (for output unpermute) and node -> table row
    node_of_slot = np.full((cfg.n_cores, cfg.npc), -1, np.int64)
    for c in range(cfg.n_cores):
        for t in range(cfg.tpc):
            node_of_slot[c, t * P:(t + 1) * P] = tiles[core_tiles[c][t]]
    row_of_node = np.full(N, -1, np.int64)
    flat_slots = node_of_slot.reshape(-1)
    real = flat_slots >= 0
    row_of_node[flat_slots[real]] = np.nonzero(real)[0]
    assert (row_of_node >= 0).all()

    # permuted, padded, transposed x (own shard per core)
    xT = np.zeros((cfg.d_in, cfg.npad), np.float32)
    xT[:, np.nonzero(real)[0]] = np.asarray(x, np.float32).T[:, flat_slots[real]]

    # packed weights; ilv: hidden col j=(c,h) maps to old col h*hid+c
    W1 = np.asarray(W1, np.float32)
    ablk_s = np.zeros((cfg.d_hid, cfg.heads), np.float32)
    ablk_d = np.zeros((cfg.d_hid, cfg.heads), np.float32)
    a_s1 = np.asarray(att_src1, np.float32)
    a_d1 = np.asarray(att_dst1, np.float32)
    for h in range(cfg.heads):
        ablk_s[h * cfg.hid:(h + 1) * cfg.hid, h] = a_s1[h]
        ablk_d[h * cfg.hid:(h + 1) * cfg.hid, h] = a_d1[h]
    Wa1 = np.concatenate([W1 @ ablk_s, W1 @ ablk_d], axis=1)
    W2 = np.asarray(W2, np.float32)
    w2s = W2 @ np.asarray(att_src2, np.float32)[0]
    w2d = W2 @ np.asarray(att_dst2, np.float32)[0]
    W2a = np.concatenate([W2, w2s[:, None], w2d[:, None]], axis=1)
    b1r = np.tile(np.asarray(b1, np.float32)[None, :], (P, 1))
    b2r = np.tile(np.asarray(b2, np.float32)[None, :], (P, 1))
    j = np.arange(cfg.d_hid)
    old = (j % cfg.heads) * cfg.hid + j // cfg.heads
    b1r = np.ascontiguousarray(b1r[:, old])
    W2a = np.ascontiguousarray(W2a[old, :])

    # iota table: value at (p, d, g) = d, [128, 128*ng_max] bf16 (same cores)
    import ml_dtypes
    iotaT = np.repeat(np.arange(P, dtype=np.float32), cfg.ng_max)
    iotaT = np.tile(iotaT[None, :], (P, 1)).astype(ml_dtypes.bfloat16)

    in_maps = []
    for c in range(cfg.n_cores):
        gi_parts, di_parts, ct_parts = [], [], []
        for t in range(cfg.tpc):
            ng = cfg.ng[t]
            epad = ng * P
            nodes = node_of_slot[c, t * P:(t + 1) * P]
            srcs = np.zeros(epad, np.int64)  # pad: row 0 (finite data)
            dloc = np.full(epad, 255, np.int64)  # pad: no dst
            off = 0
            for d in range(P):
                n = nodes[d]
                if n >= 0:
                    k = deg[n]
                    srcs[off:off + k] = row_of_node[
                        sorted_src[starts[n]:starts[n] + k]]
                    dloc[off:off + k] = d
                    off += k
            assert off == tile_ne[c, t]
            gi_parts.append(srcs)
            # edge e -> (partition e%128, group e//128)
            dmat = dloc.reshape(ng, P).T  # [128, ng]
            di_parts.append(dmat.astype(np.float32))
            ct = (dloc[None, :] == np.arange(P)[:, None])  # [128 d, epad]
            ct_parts.append(ct.astype(np.float32))
        gi = _wrap_idx(np.concatenate(gi_parts))
        dstidx = np.concatenate(di_parts, axis=1).astype(ml_dtypes.bfloat16)
        ct2 = np.concatenate(ct_parts, axis=1).astype(ml_dtypes.bfloat16)
        in_maps.append({
            "xTo": np.ascontiguousarray(
                xT[:, c * cfg.npc:(c + 1) * cfg.npc]).astype(ml_dtypes.bfloat16),
            "W1": W1.astype(ml_dtypes.bfloat16),
            "Wa1": Wa1.astype(ml_dtypes.bfloat16),
            "W2a": W2a.astype(ml_dtypes.bfloat16),
            "b1r": b1r.astype(ml_dtypes.bfloat16), "b2r": b2r,
            "gi": gi, "dstidx": dstidx, "ct2": ct2, "iotaT": iotaT,
        })
    return in_maps, node_of_slot


def build_program(cfg: Cfg) -> bass.Bass:
    import concourse.bacc as bacc
    nc = bacc.Bacc("TRN2", target_bir_lowering=False, num_devices=cfg.n_cores)
    DH, HD, DO = cfg.d_hid, cfg.heads, cfg.d_out
    KT = cfg.d_in // P
    K2 = DH // P
    NIDX = P * cfg.sum_ng
    NGM = cfg.ng_max

    # ---- DRAM ----
    xTo = nc.dram_tensor("xTo", [cfg.d_in, cfg.npc], BF16, kind="ExternalInput")
    W1 = nc.dram_tensor("W1", [cfg.d_in, DH], BF16, kind="ExternalInput")
    Wa1 = nc.dram_tensor("Wa1", [cfg.d_in, 2 * HD], BF16, kind="ExternalInput")
    b1r = nc.dram_tensor("b1r", [P, DH], BF16, kind="ExternalInput")
    W2a = nc.dram_tensor("W2a", [DH, DO + 2], BF16, kind="ExternalInput")
    b2r = nc.dram_tensor("b2r", [P, DO], F32, kind="ExternalInput")
    gi = nc.dram_tensor("gi", [P, NIDX // 16], I16, kind="ExternalInput")
    dstidx = nc.dram_tensor("dstidx", [P, cfg.sum_ng], BF16,
                            kind="ExternalInput")
    ct2 = nc.dram_tensor("ct2", [P, NIDX], BF16, kind="ExternalInput")
    iotaT = nc.dram_tensor("iotaT", [P, P * NGM], BF16, kind="ExternalInput")
    out = nc.dram_tensor("out", [cfg.npc, DO], F32, kind="ExternalOutput")

    haug = nc.dram_tensor("haug", [cfg.npad, cfg.rw1], BF16,
                          addr_space="Shared" if cfg.collective else "Local")
    haug_own = nc.dram_tensor("haug_own", [cfg.npc, cfg.rw1], BF16)
    h2own = nc.dram_tensor("h2own", [cfg.npc, cfg.rw2], BF16)
    h2all = nc.dram_tensor("h2all", [cfg.npad, cfg.rw2], BF16,
                           addr_space="Shared" if cfg.collective else "Local")

    from contextlib import ExitStack
    with tile.TileContext(nc) as tc, ExitStack() as st:
        cst = st.enter_context(tc.tile_pool(name="cst", bufs=1))
        psB_p = st.enter_context(tc.tile_pool(name="psB", bufs=3, space="PSUM"))
        psS_p = st.enter_context(tc.tile_pool(name="psS", bufs=3, space="PSUM"))
        psT_p = st.enter_context(tc.tile_pool(name="psT", bufs=2, space="PSUM"))
        hg_p = st.enter_context(tc.tile_pool(name="hg", bufs=3))
        hg2_p = st.enter_context(tc.tile_pool(name="hg2", bufs=4))
        sm_p = st.enter_context(tc.tile_pool(name="sm", bufs=6))
        big_p = st.enter_context(tc.tile_pool(name="big", bufs=2))
        hsb_p = st.enter_context(tc.tile_pool(name="hsb", bufs=3))
        out_p = st.enter_context(tc.tile_pool(name="outp", bufs=4))

        # ---- constants ----
        w1sb = cst.tile([P, KT, DH], BF16)
        wa1sb = cst.tile([P, KT, 2 * HD], BF16)
        w2sb = cst.tile([P, K2, DO + 2], BF16)
        b1sb = cst.tile([P, DH], BF16)
        b2sb = cst.tile([P, DO], F32)
        gisb = cst.tile([P, NIDX // 16], I16)
        disb = cst.tile([P, cfg.sum_ng], BF16)
        iosb = cst.tile([P, P, NGM], BF16)
        ident = cst.tile([P, P], F32)
        identb = cst.tile([P, P], BF16)
        ad_bf = cst.tile([P, cfg.tpc, HD], BF16)
        ad2_bf = cst.tile([P, cfg.tpc, 1], BF16)
        c2all = cst.tile([P, P, cfg.sum_ng], BF16)  # [e-part, d, g] per tile
        ctall = cst.tile([P, NIDX], BF16)  # [d-part, e] per tile, concat
        xosb = cst.tile([P, KT, cfg.npc], BF16)
        # x + weights first (phase A inputs), then edge-phase tables
        for k in range(KT):
            nc.sync.dma_start(out=xosb[:, k, :], in_=xTo[k * P:(k + 1) * P, :])
            nc.sync.dma_start(out=w1sb[:, k, :], in_=W1[k * P:(k + 1) * P, :])
            nc.sync.dma_start(out=wa1sb[:, k, :],
                              in_=Wa1[k * P:(k + 1) * P, :])
        for k in range(K2):
            nc.sync.dma_start(out=w2sb[:, k, :], in_=W2a[k * P:(k + 1) * P, :])
        nc.sync.dma_start(out=b1sb[:], in_=b1r[:])
        nc.sync.dma_start(out=b2sb[:], in_=b2r[:])
        nc.sync.dma_start(out=gisb[:], in_=gi[:])
        nc.sync.dma_start(out=disb[:], in_=dstidx[:])
        nc.sync.dma_start(out=iosb[:], in_=iotaT[:])
        make_identity(nc, ident[:])
        make_identity(nc, identb[:])

        # ---- Phase A: own H tiles + shared table ----
        for t in range(cfg.tpc):
            lt = xosb[:, :, t * P:(t + 1) * P]
            ph = psB_p.tile([P, DH], F32, tag="big")
            pa_t = psS_p.tile([P, 256], F32, tag="sm")
            pa = pa_t[:, :2 * HD]
            for k in range(KT):
                nc.tensor.matmul(ph[:], lt[:, k, :], w1sb[:, k, :],
                                 start=(k == 0), stop=(k == KT - 1))
            for k in range(KT):
                nc.tensor.matmul(pa[:], lt[:, k, :], wa1sb[:, k, :],
                                 start=(k == 0), stop=(k == KT - 1))
            hs = hsb_p.tile([P, cfg.rw1], BF16, tag="hsb")
            # ilv: table col j=(c,h); psum col (h,c)
            hsr = hs[:, :DH].rearrange("p (c h) -> p h c", h=HD)
            phr = ph[:].rearrange("p (h c) -> p h c", h=HD)
            half = cfg.hid // 2
            nc.vector.tensor_copy(hsr[:, :, :half], phr[:, :, :half])
            nc.scalar.copy(hsr[:, :, half:], phr[:, :, half:])
            nc.scalar.copy(hs[:, DH:DH + HD], pa[:, :HD])
            nc.vector.memset(hs[:, DH + HD:], 0.0)
            nc.scalar.copy(ad_bf[:, t, :], pa[:, HD:2 * HD])
            hdst = haug_own if cfg.collective else haug
            nc.sync.dma_start(out=hdst[t * P:(t + 1) * P, :], in_=hs[:])
        if cfg.collective:
            nc.gpsimd.collective_compute(
                "AllGather", mybir.AluOpType.bypass,
                ins=[haug_own[:]], outs=[haug[:]],
                replica_groups=[list(range(cfg.n_cores))])

        # ---- build all C2 one-hots (runs during phase A) ----
        g_off = 0
        for t in range(cfg.tpc):
            ng = cfg.ng[t]
            nc.vector.tensor_tensor(
                out=c2all[:, :, g_off:g_off + ng],
                in0=disb[:, g_off:g_off + ng].unsqueeze(1)
                    .broadcast_to([P, P, ng]),
                in1=iosb[:, :, :ng],
                op=mybir.AluOpType.is_equal)
            g_off += ng

        # ---- alpha_dst edge-broadcasts (PE); ct2 loads chunked per tile.
        # Only early tiles' results are needed at B start: defer the rest
        # into the B pipeline where the DMA has slack.
        ade_sb = cst.tile([P, cfg.sum_ng, HD], BF16)
        offs0 = np.cumsum([0] + cfg.ng).tolist()

        def ade_pre(t):
            ng = cfg.ng[t]
            e_off = P * offs0[t]
            nc.sync.dma_start(out=ctall[:, e_off:e_off + P * ng],
                              in_=ct2[:, e_off:e_off + P * ng])
            pad_e_t = psS_p.tile([P, 256], F32, tag="sm")
            for g in range(ng):
                nc.tensor.matmul(pad_e_t[:, g * HD:(g + 1) * HD],
                                 ctall[:, e_off + g * P:e_off + (g + 1) * P],
                                 ad_bf[:, t, :], start=True, stop=True)
            nc.scalar.copy(
                ade_sb[:, offs0[t]:offs0[t] + ng, :],
                pad_e_t[:, :ng * HD].rearrange("p (g h) -> p g h", h=HD))

        ADE_PRE = 4
        for t in range(ADE_PRE):
            ade_pre(t)

        doB = "B" in cfg.phases
        doC = "C" in cfg.phases
        doD = "D" in cfg.phases
        offs = np.cumsum([0] + cfg.ng).tolist()
        CSPL = 48  # mul column split: c < CSPL on DVE, rest on Pool

        # ---- Phase B+C, software-pipelined: front(t) then back(t-1) ----
        def b_front(t):
            ng = cfg.ng[t]
            g_off = offs[t]
            hg = hg_p.tile([P, NGM, cfg.rw1], BF16, tag="hg")
            hgv = hg[:, :ng, :]
            c2t = c2all[:, :, g_off:g_off + ng]
            hgm = hgv[:, :, :DH].rearrange("p g (c h) -> p g c h", h=HD)
            psD_t = psS_p.tile([P, 256], F32, tag="sm")
            psD = psD_t[:, :4 * HD]
            psY = psB_p.tile([P, DH], F32, tag="big")
            nsp = 2
            cuts = [min(ng, (ng + nsp - 1) // nsp * i) for i in range(nsp + 1)]
            parts = [(a, b) for a, b in zip(cuts, cuts[1:]) if b > a]
            # part-gathers into one buffer: earlier parts' compute overlaps
            # later parts' transfers (region-precise deps)
            for hi_, (lo, hi) in enumerate(parts):
                nh = hi - lo
                e_off = P * (g_off + lo)
                nidx = P * nh
                nc.gpsimd.dma_gather(
                    out_ap=hgv[:, lo:hi, :], in_ap=haug[:, :],
                    idxs_ap=gisb[:, e_off // 16:(e_off + nidx) // 16],
                    num_idxs=nidx, num_idxs_reg=nidx, elem_size=cfg.rw1,
                    single_packet=False)
            for hi_, (lo, hi) in enumerate(parts):
                nh = hi - lo
                ex = hgv[:, lo:hi, DH:DH + HD]
                nc.vector.tensor_tensor(
                    out=ex, in0=ex,
                    in1=ade_sb[:, g_off + lo:g_off + hi, :],
                    op=mybir.AluOpType.add)
                # exp(lrelu(x)) == max(exp(x), exp(slope*x)) exactly
                e2 = sm_p.tile([P, NGM, HD], BF16, tag="neg")
                nc.scalar.activation(e2[:, :nh, :], ex,
                                     mybir.ActivationFunctionType.Exp,
                                     scale=NEG_SLOPE)
                nc.scalar.activation(ex, ex, mybir.ActivationFunctionType.Exp)
                nc.vector.tensor_tensor(out=ex, in0=ex, in1=e2[:, :nh, :],
                                        op=mybir.AluOpType.max)
                # denominators: separate col-block per half (summed in back)
                for g in range(lo, hi):
                    nc.tensor.matmul(psD[:, hi_ * HD:(hi_ + 1) * HD],
                                     c2t[:, :, g], hgv[:, g, DH:DH + HD],
                                     start=(g == lo), stop=(g == hi - 1),
                                     skip_group_check=True)
                nc.vector.tensor_tensor(
                    out=hgm[:, lo:hi, :, :], in0=hgm[:, lo:hi, :, :],
                    in1=hgv[:, lo:hi, DH:DH + HD].unsqueeze(2)
                        .broadcast_to([P, nh, cfg.hid, HD]),
                    op=mybir.AluOpType.mult)
                for g in range(lo, hi):
                    nc.tensor.matmul(psY[:], c2t[:, :, g], hgv[:, g, :DH],
                                     start=(g == 0), stop=(g == ng - 1),
                                     skip_group_check=True)
            return psY, psD, len(parts)

        def b_back(t, psY, psD, nparts):
            den = sm_p.tile([P, HD], F32, tag="den")
            nc.vector.tensor_scalar_add(out=den[:], in0=psD[:, :HD],
                                        scalar1=0.0)
            for q in range(1, nparts):
                nc.vector.tensor_tensor(out=den[:], in0=den[:],
                                        in1=psD[:, q * HD:(q + 1) * HD],
                                        op=mybir.AluOpType.add)
            nc.vector.tensor_scalar_max(out=den[:], in0=den[:], scalar1=1e-30)
            rec = sm_p.tile([P, HD], F32, tag="rec")
            nc.vector.reciprocal(rec[:], den[:])
            y = big_p.tile([P, DH], BF16, tag="y")
            nc.vector.tensor_tensor(
                out=y[:].rearrange("p (c h) -> p c h", h=HD),
                in0=psY[:].rearrange("p (c h) -> p c h", h=HD),
                in1=rec[:].unsqueeze(1).broadcast_to([P, cfg.hid, HD]),
                op=mybir.AluOpType.mult)
            nc.vector.tensor_add(y[:], y[:], b1sb[:])
            tneg = big_p.tile([P, DH], BF16, tag="tneg")
            nc.scalar.activation(tneg[:], y[:],
                                 mybir.ActivationFunctionType.Relu, scale=-1.0)
            nc.scalar.activation(y[:], y[:],
                                 mybir.ActivationFunctionType.Relu)
            nc.scalar.activation(tneg[:], tneg[:],
                                 mybir.ActivationFunctionType.Exp, scale=-1.0)
            nc.vector.scalar_tensor_tensor(
                out=y[:], in0=tneg[:], scalar=-1.0, in1=y[:],
                op0=mybir.AluOpType.add, op1=mybir.AluOpType.add)
            if not doC:
                return
            yT = big_p.tile([P, K2, P], BF16, tag="yT")
            for k in range(K2):
                pt = psT_p.tile([P, P], BF16, tag="pt")
                nc.tensor.transpose(pt[:], y[:, k * P:(k + 1) * P], identb[:])
                nc.scalar.copy(yT[:, k, :], pt[:])
            p2_t = psS_p.tile([P, 256], F32, tag="sm")
            p2 = p2_t[:, :DO + 2]
            for k in range(K2):
                nc.tensor.matmul(p2[:], yT[:, k, :], w2sb[:, k, :],
                                 start=(k == 0), stop=(k == K2 - 1))
            h2sb = out_p.tile([P, cfg.rw2], BF16, tag="h2sb")
            nc.scalar.copy(h2sb[:, :DO + 1], p2[:, :DO + 1])
            nc.vector.memset(h2sb[:, DO + 1:], 0.0)
            nc.scalar.copy(ad2_bf[:, t, :], p2[:, DO + 1:DO + 2])
            h2dst = h2own if cfg.collective else h2all
            nc.sync.dma_start(out=h2dst[t * P:(t + 1) * P, :], in_=h2sb[:])

        if doB:
            state = {}
            for t in range(cfg.tpc):
                if ADE_PRE + t < cfg.tpc:
                    ade_pre(ADE_PRE + t)
                state[t] = b_front(t)
                if t >= 1:
                    b_back(t - 1, *state.pop(t - 1))
            b_back(cfg.tpc - 1, *state.pop(cfg.tpc - 1))

        # ---- AllGather layer-2 table ----
        if doC and cfg.collective:
            nc.gpsimd.collective_compute(
                "AllGather", mybir.AluOpType.bypass,
                ins=[h2own[:]], outs=[h2all[:]],
                replica_groups=[list(range(cfg.n_cores))])

        # ---- Phase D, software-pipelined ----
        def d_front(t):
            ng = cfg.ng[t]
            g_off = offs[t]
            hg2 = hg2_p.tile([P, NGM, cfg.rw2], BF16, tag="hg2")
            hg2v = hg2[:, :ng, :]
            c2t = c2all[:, :, g_off:g_off + ng]
            pad2_t = psS_p.tile([P, 256], F32, tag="sm")
            pad2 = pad2_t[:, :NGM]
            psO = psB_p.tile([P, DH], F32, tag="big")
            ngh = (ng + 1) // 2
            for lo, hi in ((0, ngh), (ngh, ng)):
                nh = hi - lo
                e_off = P * (g_off + lo)
                nidx = P * nh
                nc.gpsimd.dma_gather(
                    out_ap=hg2v[:, lo:hi, :], in_ap=h2all[:, :],
                    idxs_ap=gisb[:, e_off // 16:(e_off + nidx) // 16],
                    num_idxs=nidx, num_idxs_reg=nidx, elem_size=cfg.rw2,
                    single_packet=False)
            for g in range(ng):
                nc.tensor.matmul(
                    pad2[:, g:g + 1],
                    ctall[:, P * (g_off + g):P * (g_off + g + 1)],
                    ad2_bf[:, t, :], start=True, stop=True)
            for lo, hi in ((0, ngh), (ngh, ng)):
                nh = hi - lo
                ex2 = hg2v[:, lo:hi, DO:DO + 1]
                nc.vector.tensor_tensor(
                    out=ex2, in0=ex2, in1=pad2[:, lo:hi].unsqueeze(2),
                    op=mybir.AluOpType.add)
                e22 = sm_p.tile([P, NGM, 1], BF16, tag="neg2")
                nc.scalar.activation(e22[:, :nh, :], ex2,
                                     mybir.ActivationFunctionType.Exp,
                                     scale=NEG_SLOPE)
                nc.scalar.activation(ex2, ex2,
                                     mybir.ActivationFunctionType.Exp)
                nc.vector.tensor_tensor(out=ex2, in0=ex2,
                                        in1=e22[:, :nh, :],
                                        op=mybir.AluOpType.max)
                nc.vector.tensor_tensor(
                    out=hg2v[:, lo:hi, :DO], in0=hg2v[:, lo:hi, :DO],
                    in1=ex2.broadcast_to([P, nh, DO]),
                    op=mybir.AluOpType.mult)
                for g in range(lo, hi):
                    nc.tensor.matmul(psO[:, :DO + 1], c2t[:, :, g],
                                     hg2v[:, g, :DO + 1],
                                     start=(g == 0), stop=(g == ng - 1),
                                     skip_group_check=True)
            return (psO,)

        def d_back(t, psO):
            den2 = sm_p.tile([P, 1], F32, tag="den2")
            nc.vector.tensor_scalar_max(out=den2[:], in0=psO[:, DO:DO + 1],
                                        scalar1=1e-30)
            rec2 = sm_p.tile([P, 1], F32, tag="rec2")
            nc.vector.reciprocal(rec2[:], den2[:])
            osb = out_p.tile([P, DO], F32, tag="osb")
            nc.vector.tensor_tensor(
                out=osb[:], in0=psO[:, :DO],
                in1=rec2[:].broadcast_to([P, DO]),
                op=mybir.AluOpType.mult)
            nc.vector.tensor_add(osb[:], osb[:], b2sb[:])
            nc.sync.dma_start(out=out[t * P:(t + 1) * P, :], in_=osb[:])

        if doB and doC and doD:
            state = {}
            for t in range(cfg.tpc):
                state[t] = d_front(t)
                if t >= 1:
                    d_back(t - 1, *state.pop(t - 1))
            d_back(cfg.tpc - 1, *state.pop(cfg.tpc - 1))

    nc.compile()
    return nc


def default_cfg() -> Cfg:
    return Cfg()


def run(inputs: dict, cfg: Cfg | None = None, **run_kwargs):
    cfg = cfg or default_cfg()
    in_maps, node_of_slot = preprocess(cfg, **inputs)
    nc = build_program(cfg)
    res = run_bass_kernel_spmd(nc, in_maps, list(range(cfg.n_cores)),
                               **run_kwargs)
    outs = np.concatenate([res.results[c]["out"] for c in range(cfg.n_cores)],
                          axis=0)
    full = np.zeros((cfg.n_nodes, cfg.d_out), np.float32)
    flat = node_of_slot.reshape(-1)
    real = flat >= 0
    full[flat[real]] = outs[real]
    return full, res


def kernel(**inputs) -> np.ndarray:
    out, _ = run(inputs)
    return out



# revision 10
# speedup vs baseline: 1.1272x; 1.1272x over previous
"""Bass/Trainium2 kernel v3 for 2-layer GAT (nn_GAT_48919677501958).

Contract: kernel(**inputs) takes FULL unsharded numpy inputs, returns the
FULL [10000, 40] float32 output.

v3 strategy (vs v2): split-table pipelining + fp8 one-hots.
  - Nodes are dealt to 8 cores x 10 tiles of 128 dsts as in v2.  The shared
    node tables (layer-1 rows `haug`, layer-2 rows `h2tab`) are laid out in
    TWO blocks: block-a = every core's tiles [0,K), block-b = tiles [K,10).
    Each block is AllGather'ed separately (into slices of one tensor), so
      * B-phase gathers of edges with early srcs start right after the
        A-phase finishes its first K tiles (instead of after all of A), and
      * D-phase gathers of early-src edges run while the B pipeline is still
        draining, removing the B->D DMA idle window.
  - Per (core, tile) the edge list is split into an a-range (early srcs,
    gathered from block-a only) and a b-range (everything else, gathered
    from the full table).  Ranges are group(128)-aligned and core-uniform
    (SPMD); spill edges from a go to b, pad chosen to minimize slots.
  - ct2 (the [dst, edge] one-hot for PE broadcasts) is fp8 (exact 0/1):
    half the DMA bytes and SBUF footprint; matmul lhsT fp8 x rhs bf16 is
    exact on TRN2 (verified).
  - haug rows are written as 520 used elems (no pad memset / pad write).
  - Small constants are packed host-side into one DRAM blob -> one DMACopy.
  - D-phase gathers are 4 big chunks (2 per half) instead of 20 per-tile
    gathers: saves ~16us of Pool-engine SWDGE fixed cost.
"""

from dataclasses import dataclass, field

import numpy as np

import concourse.bass as bass
import concourse.mybir as mybir
import concourse.tile as tile
from concourse.bass_utils import run_bass_kernel_spmd
from concourse.masks import make_identity

F32 = mybir.dt.float32
BF16 = mybir.dt.bfloat16
FP8 = mybir.dt.float8e4
I16 = mybir.dt.int16

NEG_SLOPE = 0.2
P = 128


@dataclass
class Cfg:
    n_nodes: int = 10000
    n_cores: int = 8
    tpc: int = 10
    ksplit: int = 6  # tiles [0,K) form table block-a
    d_in: int = 256
    hid: int = 64
    heads: int = 8
    d_out: int = 40
    ga: list[int] = field(default_factory=list)  # a-range groups per tile
    gb: list[int] = field(default_factory=list)  # b-range groups per tile
    collective: bool = True
    phases: str = "ABCD"

    @property
    def npc(self):
        return self.tpc * P

    @property
    def npad(self):
        return self.n_cores * self.npc

    @property
    def na_rows(self):  # rows in table block-a
        return self.n_cores * self.ksplit * P

    @property
    def d_hid(self):
        return self.hid * self.heads

    @property
    def rw1(self):  # table-1 row elems: 512 h + 8 alpha_src + pad (256B rule)
        return 640

    @property
    def rw1u(self):  # used elems of a table-1 row
        return self.d_hid + self.heads

    @property
    def rw2(self):  # table-2 row elems: 40 h2 + 1 alpha_src2 + pad
        return 128

    @property
    def sum_ng(self):
        return sum(self.ga) + sum(self.gb)

    @property
    def ng_max(self):
        return max(max(self.ga, default=1), max(self.gb, default=1), 1)

    # global column offsets: [t0a..t9a | t0b..t9b]
    def offs_a(self, t):
        return sum(self.ga[:t])

    def offs_b(self, t):
        return sum(self.ga) + sum(self.gb[:t])


def _wrap_idx(flat: np.ndarray) -> np.ndarray:
    """dma_gather index layout: position i at [i % 16, i // 16], replicated
    across the 8 GpSimd-core stripes of 16 partitions each."""
    assert flat.size % 16 == 0
    w = np.ascontiguousarray(flat.reshape(-1, 16).T).astype(np.int16)
    return np.tile(w, (8, 1))


def preprocess(cfg: Cfg, x, edge_index, W1, att_src1, att_dst1, b1, W2,
               att_src2, att_dst2, b2):
    import ml_dtypes
    N = cfg.n_nodes
    K = cfg.ksplit
    src = np.concatenate([np.asarray(edge_index[0], np.int64), np.arange(N)])
    dst = np.concatenate([np.asarray(edge_index[1], np.int64), np.arange(N)])
    deg = np.bincount(dst, minlength=N)

    # CSR by dst
    order_e = np.argsort(dst, kind="stable")
    sorted_src = src[order_e]
    starts = np.zeros(N + 1, np.int64)
    np.cumsum(deg, out=starts[1:])

    # deal degree-sorted nodes round-robin over cores, then pack tiles
    # toward weighted edge targets (small first/last tiles).
    node_order = np.argsort(-deg, kind="stable")
    n_tiles = cfg.n_cores * cfg.tpc
    tiles = np.full((n_tiles, P), -1, np.int64)
    wts = np.array([0.42] + [1.13] * (cfg.tpc - 3) + [0.82, 0.42])

    for c in range(cfg.n_cores):
        mine = node_order[c::cfg.n_cores]
        tgt = wts / wts.sum() * deg[mine].sum()
        cur = np.zeros(cfg.tpc)
        cnt = np.zeros(cfg.tpc, np.int64)
        cap = 18 * P - 16
        for n in mine:
            free = cnt < P
            ok = free & (cur + deg[n] <= cap)
            pick = ok if ok.any() else free
            t = int(np.argmax(np.where(pick, tgt - cur, -np.inf)))
            tiles[c * cfg.tpc + t, cnt[t]] = n
            cur[t] += deg[n]
            cnt[t] += 1

    # slot -> node map and node -> (core, tile, part) slot
    node_of_slot = np.full((cfg.n_cores, cfg.npc), -1, np.int64)
    for c in range(cfg.n_cores):
        for t in range(cfg.tpc):
            node_of_slot[c, t * P:(t + 1) * P] = tiles[c * cfg.tpc + t]
    flat_slots = node_of_slot.reshape(-1)
    real = flat_slots >= 0
    slot_of_node = np.full(N, -1, np.int64)
    slot_of_node[flat_slots[real]] = np.nonzero(real)[0]
    assert (slot_of_node >= 0).all()

    # split-layout table row of a slot: block-a rows first
    def table_row(slot):
        c, r = slot // cfg.npc, slot % cfg.npc
        t, p = r // P, r % P
        if t < K:
            return c * (K * P) + t * P + p
        return cfg.na_rows + c * ((cfg.tpc - K) * P) + (t - K) * P + p

    slot_arr = np.arange(cfg.npad)
    c_arr, r_arr = slot_arr // cfg.npc, slot_arr % cfg.npc
    t_arr, p_arr = r_arr // P, r_arr % P
    trow_of_slot = np.where(
        t_arr < K, c_arr * (K * P) + t_arr * P + p_arr,
        cfg.na_rows + c_arr * ((cfg.tpc - K) * P) + (t_arr - K) * P + p_arr)
    early_of_slot = t_arr < K

    # per (core, tile): edge lists (dst-major), split early/late
    ea_list = [[None] * cfg.tpc for _ in range(cfg.n_cores)]
    eb_list = [[None] * cfg.tpc for _ in range(cfg.n_cores)]
    for c in range(cfg.n_cores):
        for t in range(cfg.tpc):
            rows_e, dloc_e, rows_l, dloc_l = [], [], [], []
            for d in range(P):
                n = node_of_slot[c, t * P + d]
                if n < 0:
                    continue
                ss = sorted_src[starts[n]:starts[n] + deg[n]]
                sl = slot_of_node[ss]
                er = early_of_slot[sl]
                tr = trow_of_slot[sl]
                rows_e.append(tr[er])
                dloc_e.append(np.full(int(er.sum()), d, np.int64))
                rows_l.append(tr[~er])
                dloc_l.append(np.full(int((~er).sum()), d, np.int64))
            ea_list[c][t] = (np.concatenate(rows_e), np.concatenate(dloc_e))
            eb_list[c][t] = (np.concatenate(rows_l), np.concatenate(dloc_l))

    # choose ga[t] minimizing total pad slots; spill a-extras into b
    cfg.ga, cfg.gb = [], []
    for t in range(cfg.tpc):
        ea = np.array([ea_list[c][t][0].size for c in range(cfg.n_cores)])
        ne = ea + np.array([eb_list[c][t][0].size
                            for c in range(cfg.n_cores)])
        best = None
        for g in range(0, int(ea.max()) // P + 1):
            placed = np.minimum(ea, g * P)
            gbt = int(np.ceil((ne - placed).max() / P)) if (ne - placed).max() \
                else 0
            pad = (g * P - placed).sum() + (gbt * P - (ne - placed)).sum()
            if best is None or pad <= best[0]:  # tie-break: larger a-range
                best = (pad, g, gbt)
        cfg.ga.append(best[1])
        cfg.gb.append(best[2])

    # build per-core slot-ordered edge arrays in global column order
    in_maps = []
    ablk_s = np.zeros((cfg.d_hid, cfg.heads), np.float32)
    ablk_d = np.zeros((cfg.d_hid, cfg.heads), np.float32)
    a_s1 = np.asarray(att_src1, np.float32)
    a_d1 = np.asarray(att_dst1, np.float32)
    for h in range(cfg.heads):
        ablk_s[h * cfg.hid:(h + 1) * cfg.hid, h] = a_s1[h]
        ablk_d[h * cfg.hid:(h + 1) * cfg.hid, h] = a_d1[h]
    W1 = np.asarray(W1, np.float32)
    Wa1 = np.concatenate([W1 @ ablk_s, W1 @ ablk_d], axis=1)
    W2 = np.asarray(W2, np.float32)
    w2s = W2 @ np.asarray(att_src2, np.float32)[0]
    w2d = W2 @ np.asarray(att_dst2, np.float32)[0]
    W2a = np.concatenate([W2, w2s[:, None], w2d[:, None]], axis=1)
    b1r = np.tile(np.asarray(b1, np.float32)[None, :], (P, 1))
    b2r = np.tile(np.asarray(b2, np.float32)[None, :], (P, 1))
    j = np.arange(cfg.d_hid)
    old = (j % cfg.heads) * cfg.hid + j // cfg.heads
    b1r = np.ascontiguousarray(b1r[:, old])
    W2a = np.ascontiguousarray(W2a[old, :])

    NGM = cfg.ng_max
    iotaT = np.repeat(np.arange(P, dtype=np.float32), NGM)
    iotaT = np.tile(iotaT[None, :], (P, 1))

    # permuted, padded, transposed x (own shard per core)
    xT = np.zeros((cfg.d_in, cfg.npad), np.float32)
    xT[:, np.nonzero(real)[0]] = np.asarray(x, np.float32).T[:, flat_slots[real]]

    KT = cfg.d_in // P
    K2 = cfg.d_hid // P
    for c in range(cfg.n_cores):
        gi_parts = [None] * (2 * cfg.tpc)
        di_parts = [None] * (2 * cfg.tpc)
        for t in range(cfg.tpc):
            rows_a0, dloc_a0 = ea_list[c][t]
            rows_b0, dloc_b0 = eb_list[c][t]
            na = cfg.ga[t] * P
            # a-slots: first na early edges (pad if short)
            rows_a = np.zeros(na, np.int64)
            dloc_a = np.full(na, 255, np.int64)
            take = min(na, rows_a0.size)
            rows_a[:take] = rows_a0[:take]
            dloc_a[:take] = dloc_a0[:take]
            # b-slots: early spill + all late
            nb = cfg.gb[t] * P
            rows_b = np.zeros(nb, np.int64)
            dloc_b = np.full(nb, 255, np.int64)
            sp_r = np.concatenate([rows_a0[take:], rows_b0])
            sp_d = np.concatenate([dloc_a0[take:], dloc_b0])
            assert sp_r.size <= nb
            rows_b[:sp_r.size] = sp_r
            dloc_b[:sp_d.size] = sp_d
            gi_parts[t] = rows_a
            gi_parts[cfg.tpc + t] = rows_b
            di_parts[t] = dloc_a.reshape(cfg.ga[t], P).T if cfg.ga[t] else \
                np.zeros((P, 0))
            di_parts[cfg.tpc + t] = dloc_b.reshape(cfg.gb[t], P).T \
                if cfg.gb[t] else np.zeros((P, 0))
        gi = _wrap_idx(np.concatenate(gi_parts))
        dstidx = np.concatenate(di_parts, axis=1).astype(np.float32)
        ct2 = (dstidx[None, :, :] == np.arange(P)[:, None, None])
        # ct2 as [d, edge] with edge = (p, g) -> column g*P + p? No: edge
        # column layout must match matmul lhsT slicing: per group g we take
        # ctall[:, (col_g)*P:(col_g+1)*P] = one-hot [d, p_e]; so flatten as
        # (g, p): column index = g*P + p.
        ct2 = np.ascontiguousarray(
            ct2.transpose(0, 2, 1).reshape(P, -1)).astype(
                ml_dtypes.float8_e4m3fn)
        wpack = np.concatenate([
            W1.reshape(KT, P, cfg.d_hid).transpose(1, 0, 2).reshape(P, -1),
            Wa1.reshape(KT, P, 2 * cfg.heads).transpose(1, 0, 2).reshape(P, -1),
            W2a.reshape(K2, P, cfg.d_out + 2).transpose(1, 0, 2).reshape(P, -1),
            b1r,
            dstidx,
            iotaT[:, :P * NGM],
        ], axis=1).astype(ml_dtypes.bfloat16)
        in_maps.append({
            "xTo": np.ascontiguousarray(
                xT[:, c * cfg.npc:(c + 1) * cfg.npc]).astype(
                    ml_dtypes.bfloat16),
            "wpack": wpack,
            "b2r": b2r.astype(np.float32),
            "gi": gi, "ct2": ct2,
        })
    return in_maps, node_of_slot


def build_program(cfg: Cfg) -> bass.Bass:
    import concourse.bacc as bacc
    nc = bacc.Bacc("TRN2", target_bir_lowering=False, num_devices=cfg.n_cores)
    DH, HD, DO = cfg.d_hid, cfg.heads, cfg.d_out
    K = cfg.ksplit
    KT = cfg.d_in // P
    K2 = DH // P
    NIDX = P * cfg.sum_ng
    NGM = cfg.ng_max
    NA = cfg.na_rows
    RW1U = cfg.rw1u  # 520
    WC_W1 = KT * DH
    WC_WA = KT * 2 * HD
    WC_W2 = K2 * (DO + 2)
    WCOLS = WC_W1 + WC_WA + WC_W2 + DH + cfg.sum_ng + P * NGM
    o_wa = WC_W1
    o_w2 = o_wa + WC_WA
    o_b1 = o_w2 + WC_W2
    o_di = o_b1 + DH
    o_io = o_di + cfg.sum_ng

    # ---- DRAM ----
    xTo = nc.dram_tensor("xTo", [cfg.d_in, cfg.npc], BF16,
                         kind="ExternalInput")
    wpk = nc.dram_tensor("wpack", [P, WCOLS], BF16, kind="ExternalInput")
    b2r = nc.dram_tensor("b2r", [P, DO], F32, kind="ExternalInput")
    gi = nc.dram_tensor("gi", [P, NIDX // 16], I16, kind="ExternalInput")
    ct2 = nc.dram_tensor("ct2", [P, NIDX], FP8, kind="ExternalInput")
    out = nc.dram_tensor("out", [cfg.npc, DO], F32, kind="ExternalOutput")

    haug = nc.dram_tensor("haug", [cfg.npad, cfg.rw1], BF16,
                          addr_space="Shared" if cfg.collective else "Local")
    h2tab = nc.dram_tensor("h2tab", [cfg.npad, cfg.rw2], BF16,
                           addr_space="Shared" if cfg.collective else "Local")
    if cfg.collective:
        hoa = nc.dram_tensor("hoa", [K * P, cfg.rw1], BF16)
        hob = nc.dram_tensor("hob", [(cfg.tpc - K) * P, cfg.rw1], BF16)
        h2oa = nc.dram_tensor("h2oa", [K * P, cfg.rw2], BF16)
        h2ob = nc.dram_tensor("h2ob", [(cfg.tpc - K) * P, cfg.rw2], BF16)
    grp = [list(range(cfg.n_cores))]

    from contextlib import ExitStack
    with tile.TileContext(nc) as tc, ExitStack() as st:
        cst = st.enter_context(tc.tile_pool(name="cst", bufs=1))
        psB_p = st.enter_context(tc.tile_pool(name="psB", bufs=3,
                                              space="PSUM"))
        psS_p = st.enter_context(tc.tile_pool(name="psS", bufs=3,
                                              space="PSUM"))
        psT_p = st.enter_context(tc.tile_pool(name="psT", bufs=2,
                                              space="PSUM"))
        hga_p = st.enter_context(tc.tile_pool(name="hga", bufs=3))
        hgb_p = st.enter_context(tc.tile_pool(name="hgb", bufs=3))
        hg2_p = st.enter_context(tc.tile_pool(name="hg2", bufs=2))
        sm_p = st.enter_context(tc.tile_pool(name="sm", bufs=6))
        big_p = st.enter_context(tc.tile_pool(name="big", bufs=2))
        hsb_p = st.enter_context(tc.tile_pool(name="hsb", bufs=3))
        out_p = st.enter_context(tc.tile_pool(name="outp", bufs=4))

        # ---- constants ----
        xosb = cst.tile([P, KT, cfg.npc], BF16)
        wsb = cst.tile([P, WCOLS], BF16)
        b2sb = cst.tile([P, DO], F32)
        gisb = cst.tile([P, NIDX // 16], I16)
        ctall = cst.tile([P, NIDX], FP8)
        c2all = cst.tile([P, P, cfg.sum_ng], BF16)
        identb = cst.tile([P, P], BF16)
        ad_bf = cst.tile([P, cfg.tpc, HD], BF16)
        ad2_bf = cst.tile([P, cfg.tpc, 1], BF16)
        ade_sb = cst.tile([P, cfg.sum_ng, HD], BF16)
        ad2e_sb = cst.tile([P, cfg.sum_ng], BF16)
        osbA = cst.tile([P, cfg.tpc, DO + 1], F32)

        nc.sync.dma_start(out=xosb[:], in_=xTo[:].rearrange(
            "(k p) n -> p k n", p=P))
        nc.sync.dma_start(out=wsb[:], in_=wpk[:])
        nc.sync.dma_start(out=gisb[:], in_=gi[:])
        nc.sync.dma_start(out=b2sb[:], in_=b2r[:])
        make_identity(nc, identb[:])

        w1sb = wsb[:, :WC_W1].rearrange("p (k d) -> p k d", k=KT)
        wa1sb = wsb[:, o_wa:o_wa + WC_WA].rearrange("p (k d) -> p k d", k=KT)
        w2sb = wsb[:, o_w2:o_w2 + WC_W2].rearrange("p (k d) -> p k d", k=K2)
        b1sb = wsb[:, o_b1:o_b1 + DH]
        disb = wsb[:, o_di:o_di + cfg.sum_ng]
        iosb = wsb[:, o_io:o_io + P * NGM].rearrange("p (d g) -> p d g", d=P)

        half = cfg.hid // 2

        def a_tile(t):
            lt = xosb[:, :, t * P:(t + 1) * P]
            ph = psB_p.tile([P, DH], F32, tag="big")
            pa_t = psS_p.tile([P, 256], F32, tag="sm")
            pa = pa_t[:, :2 * HD]
            for k in range(KT):
                nc.tensor.matmul(ph[:], lt[:, k, :], w1sb[:, k, :],
                                 start=(k == 0), stop=(k == KT - 1))
            for k in range(KT):
                nc.tensor.matmul(pa[:], lt[:, k, :], wa1sb[:, k, :],
                                 start=(k == 0), stop=(k == KT - 1))
            hs = hsb_p.tile([P, RW1U], BF16, tag="hsb")
            hsr = hs[:, :DH].rearrange("p (c h) -> p h c", h=HD)
            phr = ph[:].rearrange("p (h c) -> p h c", h=HD)
            nc.vector.tensor_copy(hsr[:, :, :half], phr[:, :, :half])
            nc.scalar.copy(hsr[:, :, half:], phr[:, :, half:])
            nc.scalar.copy(hs[:, DH:DH + HD], pa[:, :HD])
            nc.scalar.copy(ad_bf[:, t, :], pa[:, HD:2 * HD])
            if cfg.collective:
                hdst = hoa if t < K else hob
                r0 = (t if t < K else t - K) * P
            else:
                hdst = haug
                r0 = t * P if t < K else NA + (t - K) * P
            nc.sync.dma_start(out=hdst[r0:r0 + P, :RW1U], in_=hs[:])

        # one-hot C2 build for tile t (both ranges) on DVE
        def c2_build(t):
            for off, ng in ((cfg.offs_a(t), cfg.ga[t]),
                            (cfg.offs_b(t), cfg.gb[t])):
                if ng == 0:
                    continue
                nc.vector.tensor_tensor(
                    out=c2all[:, :, off:off + ng],
                    in0=disb[:, off:off + ng].unsqueeze(1)
                        .broadcast_to([P, P, ng]),
                    in1=iosb[:, :, :ng],
                    op=mybir.AluOpType.is_equal)

        # ct2 chunk load + alpha_dst edge-broadcast for tile t
        def ade_pre(t):
            pad_e_t = psS_p.tile([P, 256], F32, tag="sm")
            j = 0
            for off, ng in ((cfg.offs_a(t), cfg.ga[t]),
                            (cfg.offs_b(t), cfg.gb[t])):
                if ng == 0:
                    continue
                e0 = P * off
                nc.sync.dma_start(out=ctall[:, e0:e0 + P * ng],
                                  in_=ct2[:, e0:e0 + P * ng])
                for g in range(ng):
                    nc.tensor.matmul(
                        pad_e_t[:, (j + g) * HD:(j + g + 1) * HD],
                        ctall[:, e0 + g * P:e0 + (g + 1) * P],
                        ad_bf[:, t, :], start=True, stop=True)
                nc.scalar.copy(
                    ade_sb[:, off:off + ng, :],
                    pad_e_t[:, j * HD:(j + ng) * HD]
                    .rearrange("p (g h) -> p g h", h=HD))
                j += ng

        doB = "B" in cfg.phases
        doC = "C" in cfg.phases
        doD = "D" in cfg.phases

        def gather_a(t):
            if not cfg.ga[t]:
                return None
            hga = hga_p.tile([P, max(cfg.ga), cfg.rw1], BF16, tag="hga")
            off = cfg.offs_a(t)
            e0, nidx = P * off, P * cfg.ga[t]
            nc.gpsimd.dma_gather(
                out_ap=hga[:, :cfg.ga[t], :], in_ap=haug[0:NA, :],
                idxs_ap=gisb[:, e0 // 16:(e0 + nidx) // 16],
                num_idxs=nidx, num_idxs_reg=nidx, elem_size=cfg.rw1,
                single_packet=False)
            return hga

        # ---- phase A + AllGathers; early a-gathers overlap late A tiles ----
        ADE_PRE = 3
        NPRE = 3
        pre_a = {}
        for t in range(K):
            a_tile(t)
            c2_build(t)
            if t < ADE_PRE:
                ade_pre(t)
        if cfg.collective:
            nc.gpsimd.collective_compute(
                "AllGather", mybir.AluOpType.bypass,
                ins=[hoa[:]], outs=[haug[0:NA, :]], replica_groups=grp)
        if doB:
            for t in range(NPRE):
                pre_a[t] = gather_a(t)
        for t in range(K, cfg.tpc):
            a_tile(t)
            c2_build(t)
        if cfg.collective:
            nc.gpsimd.collective_compute(
                "AllGather", mybir.AluOpType.bypass,
                ins=[hob[:]], outs=[haug[NA:, :]], replica_groups=grp)

        # ---- phase B ----
        def b_front(t):
            parts = []
            if cfg.ga[t]:
                hga = pre_a.pop(t) if t in pre_a else gather_a(t)
                parts.append((cfg.offs_a(t), cfg.ga[t], hga))
            if cfg.gb[t]:
                hgb = hgb_p.tile([P, max(cfg.gb), cfg.rw1], BF16, tag="hgb")
                off = cfg.offs_b(t)
                e0, nidx = P * off, P * cfg.gb[t]
                nc.gpsimd.dma_gather(
                    out_ap=hgb[:, :cfg.gb[t], :], in_ap=haug[:, :],
                    idxs_ap=gisb[:, e0 // 16:(e0 + nidx) // 16],
                    num_idxs=nidx, num_idxs_reg=nidx, elem_size=cfg.rw1,
                    single_packet=False)
                parts.append((cfg.offs_b(t), cfg.gb[t], hgb))
            psD_t = psS_p.tile([P, 256], F32, tag="sm")
            psY = psB_p.tile([P, DH], F32, tag="big")
            ngtot = cfg.ga[t] + cfg.gb[t]
            gdone = 0
            for pi, (off, ng, buf) in enumerate(parts):
                hgv = buf[:, :ng, :]
                ex = hgv[:, :, DH:DH + HD]
                nc.vector.tensor_tensor(
                    out=ex, in0=ex, in1=ade_sb[:, off:off + ng, :],
                    op=mybir.AluOpType.add)
                # exp(lrelu(x)) == max(exp(x), exp(slope*x)) exactly
                e2 = sm_p.tile([P, NGM, HD], BF16, tag="neg")
                nc.scalar.activation(e2[:, :ng, :], ex,
                                     mybir.ActivationFunctionType.Exp,
                                     scale=NEG_SLOPE)
                nc.scalar.activation(ex, ex, mybir.ActivationFunctionType.Exp)
                nc.vector.tensor_tensor(out=ex, in0=ex, in1=e2[:, :ng, :],
                                        op=mybir.AluOpType.max)
                # denominator: one accumulation across both parts
                for g in range(ng):
                    nc.tensor.matmul(psD_t[:, :HD],
                                     c2all[:, :, off + g],
                                     hgv[:, g, DH:DH + HD],
                                     start=(gdone + g == 0),
                                     stop=(gdone + g == ngtot - 1),
                                     skip_group_check=True)
                hgm = hgv[:, :, :DH].rearrange("p g (c h) -> p g c h", h=HD)
                nc.vector.tensor_tensor(
                    out=hgm, in0=hgm,
                    in1=ex.unsqueeze(2).broadcast_to([P, ng, cfg.hid, HD]),
                    op=mybir.AluOpType.mult)
                for g in range(ng):
                    nc.tensor.matmul(psY[:], c2all[:, :, off + g],
                                     hgv[:, g, :DH],
                                     start=(gdone + g == 0),
                                     stop=(gdone + g == ngtot - 1),
                                     skip_group_check=True)
                gdone += ng
            return psY, psD_t, len(parts)

        def b_back(t, psY, psD, nparts):
            den = sm_p.tile([P, HD], F32, tag="den")
            nc.vector.tensor_scalar_max(out=den[:], in0=psD[:, :HD],
                                        scalar1=1e-30)
            rec = sm_p.tile([P, HD], F32, tag="rec")
            nc.vector.reciprocal(rec[:], den[:])
            y = big_p.tile([P, DH], BF16, tag="y")
            nc.vector.tensor_tensor(
                out=y[:].rearrange("p (c h) -> p c h", h=HD),
                in0=psY[:].rearrange("p (c h) -> p c h", h=HD),
                in1=rec[:].unsqueeze(1).broadcast_to([P, cfg.hid, HD]),
                op=mybir.AluOpType.mult)
            nc.vector.tensor_add(y[:], y[:], b1sb[:])
            tneg = big_p.tile([P, DH], BF16, tag="tneg")
            nc.scalar.activation(tneg[:], y[:],
                                 mybir.ActivationFunctionType.Relu, scale=-1.0)
            nc.scalar.activation(y[:], y[:],
                                 mybir.ActivationFunctionType.Relu)
            nc.scalar.activation(tneg[:], tneg[:],
                                 mybir.ActivationFunctionType.Exp, scale=-1.0)
            nc.vector.scalar_tensor_tensor(
                out=y[:], in0=tneg[:], scalar=-1.0, in1=y[:],
                op0=mybir.AluOpType.add, op1=mybir.AluOpType.add)
            if not doC:
                return
            yT = big_p.tile([P, K2, P], BF16, tag="yT")
            for k in range(K2):
                pt = psT_p.tile([P, P], BF16, tag="pt")
                nc.tensor.transpose(pt[:], y[:, k * P:(k + 1) * P], identb[:])
                nc.scalar.copy(yT[:, k, :], pt[:])
            p2_t = psS_p.tile([P, 256], F32, tag="sm")
            p2 = p2_t[:, :DO + 2]
            for k in range(K2):
                nc.tensor.matmul(p2[:], yT[:, k, :], w2sb[:, k, :],
                                 start=(k == 0), stop=(k == K2 - 1))
            h2sb = out_p.tile([P, DO + 1], BF16, tag="h2sb")
            nc.scalar.copy(h2sb[:], p2[:, :DO + 1])
            nc.scalar.copy(ad2_bf[:, t, :], p2[:, DO + 1:DO + 2])
            if cfg.collective:
                h2dst = h2oa if t < K else h2ob
                r0 = (t if t < K else t - K) * P
            else:
                h2dst = h2tab
                r0 = t * P if t < K else NA + (t - K) * P
            nc.sync.dma_start(out=h2dst[r0:r0 + P, :DO + 1], in_=h2sb[:])

        # alpha_dst2 edge-broadcast for tile t (after b_back(t))
        def pad2_pre(t):
            pq_t = psS_p.tile([P, 256], F32, tag="sm")
            j = 0
            for off, ng in ((cfg.offs_a(t), cfg.ga[t]),
                            (cfg.offs_b(t), cfg.gb[t])):
                for g in range(ng):
                    nc.tensor.matmul(
                        pq_t[:, j + g:j + g + 1],
                        ctall[:, P * (off + g):P * (off + g + 1)],
                        ad2_bf[:, t, :], start=True, stop=True)
                j += ng
            ngt = cfg.ga[t] + cfg.gb[t]
            nc.scalar.copy(ad2e_sb[:, cfg.offs_a(t):cfg.offs_a(t) + cfg.ga[t]],
                           pq_t[:, :cfg.ga[t]]) if cfg.ga[t] else None
            nc.scalar.copy(ad2e_sb[:, cfg.offs_b(t):cfg.offs_b(t) + cfg.gb[t]],
                           pq_t[:, cfg.ga[t]:ngt]) if cfg.gb[t] else None

        # ---- D gather chunks: 2 per half, split by tile ranges ----
        def d_gather(chunk_tiles, a_side, buf):
            offs = [cfg.offs_a(t) if a_side else cfg.offs_b(t)
                    for t in chunk_tiles]
            ngs = [cfg.ga[t] if a_side else cfg.gb[t] for t in chunk_tiles]
            ngsum = sum(ngs)
            if ngsum == 0:
                return
            e0 = P * offs[0]
            nidx = P * ngsum
            src = h2tab[0:NA, :] if a_side else h2tab[:, :]
            nc.gpsimd.dma_gather(
                out_ap=buf[:, :ngsum, :], in_ap=src,
                idxs_ap=gisb[:, e0 // 16:(e0 + nidx) // 16],
                num_idxs=nidx, num_idxs_reg=nidx, elem_size=cfg.rw2,
                single_packet=False)

        def d_compute(t, a_side, buf, coloff):
            """one range of tile t inside a gathered chunk buf"""
            off = cfg.offs_a(t) if a_side else cfg.offs_b(t)
            ng = cfg.ga[t] if a_side else cfg.gb[t]
            psO = None
            if ng:
                hg2v = buf[:, coloff:coloff + ng, :]
                ex2 = hg2v[:, :, DO:DO + 1]
                nc.vector.tensor_tensor(
                    out=ex2, in0=ex2,
                    in1=ad2e_sb[:, off:off + ng].unsqueeze(2),
                    op=mybir.AluOpType.add)
                e22 = sm_p.tile([P, NGM, 1], BF16, tag="neg2")
                nc.scalar.activation(e22[:, :ng, :], ex2,
                                     mybir.ActivationFunctionType.Exp,
                                     scale=NEG_SLOPE)
                nc.scalar.activation(ex2, ex2,
                                     mybir.ActivationFunctionType.Exp)
                nc.vector.tensor_tensor(out=ex2, in0=ex2, in1=e22[:, :ng, :],
                                        op=mybir.AluOpType.max)
                nc.vector.tensor_tensor(
                    out=hg2v[:, :, :DO], in0=hg2v[:, :, :DO],
                    in1=ex2.broadcast_to([P, ng, DO]),
                    op=mybir.AluOpType.mult)
                psO = psS_p.tile([P, 256], F32, tag="sm")
                for g in range(ng):
                    nc.tensor.matmul(psO[:, :DO + 1], c2all[:, :, off + g],
                                     hg2v[:, g, :DO + 1],
                                     start=(g == 0), stop=(g == ng - 1),
                                     skip_group_check=True)
            if a_side:
                if psO is None:
                    nc.vector.memset(osbA[:, t, :], 0.0)
                else:
                    nc.scalar.copy(osbA[:, t, :], psO[:, :DO + 1])
                return
            # b side: combine with osbA and finalize
            osb = out_p.tile([P, DO], F32, tag="osb")
            den2 = sm_p.tile([P, 1], F32, tag="den2")
            rec2 = sm_p.tile([P, 1], F32, tag="rec2")
            tot = sm_p.tile([P, DO], F32, tag="tot")
            if psO is None:
                nc.vector.tensor_scalar_max(
                    out=den2[:], in0=osbA[:, t, DO:DO + 1], scalar1=1e-30)
                nc.vector.tensor_scalar_add(out=tot[:], in0=osbA[:, t, :DO],
                                            scalar1=0.0)
            else:
                nc.vector.scalar_tensor_tensor(
                    out=den2[:], in0=psO[:, DO:DO + 1], scalar=1e-30,
                    in1=osbA[:, t, DO:DO + 1],
                    op0=mybir.AluOpType.max, op1=mybir.AluOpType.add)
                nc.vector.tensor_tensor(out=tot[:], in0=psO[:, :DO],
                                        in1=osbA[:, t, :DO],
                                        op=mybir.AluOpType.add)
            nc.vector.reciprocal(rec2[:], den2[:])
            nc.vector.scalar_tensor_tensor(
                out=osb[:], in0=tot[:], scalar=rec2[:, 0:1], in1=b2sb[:],
                op0=mybir.AluOpType.mult, op1=mybir.AluOpType.add)
            nc.sync.dma_start(out=out[t * P:(t + 1) * P, :], in_=osb[:])

        CH_A = [list(range(0, 5)), list(range(5, cfg.tpc))]
        CH2MAX = max(max(sum(cfg.ga[t] for t in ts) for ts in CH_A),
                     max(sum(cfg.gb[t] for t in ts) for ts in CH_A), 1)

        if doB:
            bufs2 = {}
            state = {}
            for t in range(cfg.tpc):
                if ADE_PRE + t < cfg.tpc:
                    ade_pre(ADE_PRE + t)
                state[t] = b_front(t)
                if t >= 1:
                    b_back(t - 1, *state.pop(t - 1))
                    if doD:
                        pad2_pre(t - 1)
            if doD and doC:
                # issue AG-h2-a + D-a gathers behind the last B gathers;
                # the AG's input rows (tiles 0..K-1) completed long ago.
                if cfg.collective:
                    nc.gpsimd.collective_compute(
                        "AllGather", mybir.AluOpType.bypass,
                        ins=[h2oa[:]], outs=[h2tab[0:NA, :]],
                        replica_groups=grp)
                for ci, ts in enumerate(CH_A):
                    buf = hg2_p.tile([P, CH2MAX, cfg.rw2], BF16, tag="hg2")
                    bufs2[ci] = buf
                    d_gather(ts, True, buf)
            b_back(cfg.tpc - 1, *state.pop(cfg.tpc - 1))
            if doD:
                pad2_pre(cfg.tpc - 1)
            if cfg.collective and doC:
                nc.gpsimd.collective_compute(
                    "AllGather", mybir.AluOpType.bypass,
                    ins=[h2ob[:]], outs=[h2tab[NA:, :]], replica_groups=grp)

        if doB and doC and doD:
            # D-a compute of chunk ci, then reuse its buffer for D-b chunk ci
            # (issue order read-before-write keeps the WAR dep correct).
            bufs2b = {}
            for ci, ts in enumerate(CH_A):
                co = 0
                for t in ts:
                    d_compute(t, True, bufs2[ci], co)
                    co += cfg.ga[t]
                buf = hg2_p.tile([P, CH2MAX, cfg.rw2], BF16, tag="hg2")
                bufs2b[ci] = buf
                d_gather(ts, False, buf)
            for ci, ts in enumerate(CH_A):
                co = 0
                for t in ts:
                    d_compute(t, False, bufs2b[ci], co)
                    co += cfg.gb[t]

    nc.compile()
    return nc


def default_cfg() -> Cfg:
    return Cfg()


def run(inputs: dict, cfg: Cfg | None = None, **run_kwargs):
    cfg = cfg or default_cfg()
    in_maps, node_of_slot = preprocess(cfg, **inputs)
    nc = build_program(cfg)
    res = run_bass_kernel_spmd(nc, in_maps, list(range(cfg.n_cores)),
                               **run_kwargs)
    outs = np.concatenate([res.results[c]["out"] for c in range(cfg.n_cores)],
                          axis=0)
    full = np.zeros((cfg.n_nodes, cfg.d_out), np.float32)
    flat = node_of_slot.reshape(-1)
    real = flat >= 0
    full[flat[real]] = outs[real]
    return full, res


def kernel(**inputs) -> np.ndarray:
    out, _ = run(inputs)
    return out
